# revision 1
# baseline (speedup 1.0000x reference)
"""Trainium2 Bass kernel for a dense transformer encoder layer.

Problem: B=2, S=2048, D=1024, H=16 heads (W=64), F=4096, fp32.

Sharding: 8 cores = 2 batches x 4 sequence chunks of 512 tokens. Each core
computes K/V for its batch's full sequence (replicated within its 4-core
batch group) and Q/attention/FFN for its own 512-token chunk. No collectives.

Dataflow: activations live TRANSPOSED in SBUF ([feature, token], feature on
partitions) so QKV projections, attention, output projection and both FFN
matmuls chain on the TensorEngine with no on-device transposes. The host
transposes x on the way in and the per-core 1024x512 output on the way out.

Softmax: score tiles are [key-token, query-token]. The additive -10000 mask
is folded multiplicatively into V and into the per-head Z column as
gamma_t = exp(-10000*(1-m_t)) (exactly 0/1 in fp32), so exp needs no bias
and pairs of key-chunks share one wide ACT call. The normalizer Z comes
free as a 65th gamma-column appended to each head of V (the attention-value
matmul emits it as PSUM row 64); normalization multiplies by a PE-broadcast
reciprocal row. LayerNorm statistics ride 1/D-scaled ones-column matmuls
and the affine apply is two DVE passes against PE-built rank-1 tiles.

Matmuls run in float32r (TF32-like, 4x PE throughput, ~5e-4 rel error
end to end). Set USE_F32R = False for exact-fp32 matmuls (~3x slower).
"""
import numpy as np
import concourse.bass as bass
from concourse import bacc
import concourse.mybir as mybir
import concourse.tile as tile
from concourse.bass import ts
from concourse.bass_utils import run_bass_kernel_spmd

P = 128
B, S, D, H, W, F = 2, 2048, 1024, 16, 64, 4096
DC = D // P            # 8 d-chunks
FC = F // P            # 32 f-chunks
TC = S // P            # 16 key-token chunks
SCH = 512              # tokens per core
EPS = 1e-12
SCALE = 1.0 / np.sqrt(np.float32(W))
WA = W + 1             # per-head V columns incl. ones column

F32 = mybir.dt.float32
# float32r = TF32-like PE mode (4x matmul throughput, ~1e-4 rel err).
# float32  = exact fp32 matmul (4 cycles/row).
USE_F32R = True
DT = mybir.dt.float32r if USE_F32R else F32

_cache = {}


def _layer_norm(nc, tc, pp, pp2, ppacc, onesw, invd, src, sq, dst, grow, nbrow, tag):
    """src/sq/dst: [P, DC, SCH] sbuf (feature on partitions). LN over features.
    sq = src*src comes from the caller's producing evacuation. Mean scaling
    rides the stats matmuls via the invd column. The apply is two DVE passes:
    dst = src*A - B with rank-1 A = g (x) rstd, B = g (x) u*rstd - b (x) 1
    built on the PE (grow = [1,D] gamma row, nbrow = [1,D] row of -beta)."""
    at = mybir.ActivationFunctionType
    with tc.tile_pool(name=tag, bufs=1) as pool:
        ps_u = pp.tile([1, SCH], F32, tag="ps")
        ps_v = pp.tile([1, SCH], F32, tag="ps")
        for dc in range(DC):
            nc.tensor.matmul(ps_u[:], invd[:], src[:, dc],
                             start=(dc == 0), stop=(dc == DC - 1))
        for dc in range(DC):
            nc.tensor.matmul(ps_v[:], invd[:], sq[:, dc],
                             start=(dc == 0), stop=(dc == DC - 1))
        u = pool.tile([1, SCH], DT)
        var = pool.tile([1, SCH], F32)
        sd = pool.tile([1, SCH], F32)
        rstd = pool.tile([1, SCH], DT)
        ur = pool.tile([1, SCH], DT)
        nc.vector.tensor_copy(u[:], ps_u[:])
        nc.vector.tensor_tensor(var[:], u[:], u[:], mybir.AluOpType.mult)
        nc.vector.tensor_tensor(var[:], ps_v[:], var[:], mybir.AluOpType.subtract)
        nc.scalar.activation(sd[:], var[:], at.Sqrt, bias=EPS)
        nc.vector.reciprocal(rstd[:], sd[:])
        nc.vector.tensor_tensor(ur[:], u[:], rstd[:], mybir.AluOpType.mult)
        for dc in range(DC):
            ps_a = ppacc.tile([P, SCH], F32, tag="acc")
            ps_b = pp2.tile([P, SCH], F32, tag="ps2")
            nc.tensor.matmul(ps_a[:], grow[:, ts(dc, P)], rstd[:],
                             start=True, stop=True)
            nc.tensor.matmul(ps_b[:], grow[:, ts(dc, P)], ur[:],
                             start=True, stop=False)
            nc.tensor.matmul(ps_b[:], nbrow[:, ts(dc, P)], onesw[0:1, 0:SCH],
                             start=False, stop=True)
            t = pool.tile([P, SCH], F32, tag="lnt", bufs=2)
            nc.vector.tensor_tensor(t[:], src[:, dc], ps_a[:],
                                    mybir.AluOpType.mult)
            nc.vector.tensor_tensor(dst[:, dc], t[:], ps_b[:],
                                    mybir.AluOpType.subtract)


def _build():
    at = mybir.ActivationFunctionType
    nc = bacc.Bacc("TRN2", target_bir_lowering=False)

    xT_d = nc.dram_tensor("xT", [P, DC, S], DT, kind="ExternalInput")
    xs_d = nc.dram_tensor("xs", [P, DC, SCH], DT, kind="ExternalInput")
    wq_d = nc.dram_tensor("wq", [P, DC, D], DT, kind="ExternalInput")
    wk_d = nc.dram_tensor("wk", [P, DC, D], DT, kind="ExternalInput")
    wv_d = nc.dram_tensor("wv", [P, DC, D], DT, kind="ExternalInput")
    wo_d = nc.dram_tensor("wo", [P, DC, D], DT, kind="ExternalInput")
    w1_d = nc.dram_tensor("w1", [P, DC, F], DT, kind="ExternalInput")
    w2_d = nc.dram_tensor("w2", [P, FC, D], DT, kind="ExternalInput")
    ones_d = nc.dram_tensor("ones_c", [P, 512], DT, kind="ExternalInput")
    bq_d = nc.dram_tensor("bq", [P, DC], F32, kind="ExternalInput")
    bk_d = nc.dram_tensor("bk", [P, DC], F32, kind="ExternalInput")
    bv_d = nc.dram_tensor("bvr", [1, D], DT, kind="ExternalInput")
    bo_d = nc.dram_tensor("bo", [P, DC], F32, kind="ExternalInput")
    bf1_d = nc.dram_tensor("bf1", [P, FC], F32, kind="ExternalInput")
    bf2_d = nc.dram_tensor("bf2", [P, DC], F32, kind="ExternalInput")
    g1_d = nc.dram_tensor("g1", [P, DC], F32, kind="ExternalInput")
    b1_d = nc.dram_tensor("b1", [P, DC], F32, kind="ExternalInput")
    g2_d = nc.dram_tensor("g2", [P, DC], F32, kind="ExternalInput")
    b2_d = nc.dram_tensor("b2", [P, DC], F32, kind="ExternalInput")
    gam_d = nc.dram_tensor("gam", [P, TC], F32, kind="ExternalInput")
    invd_d = nc.dram_tensor("invd", [P, 1], DT, kind="ExternalInput")
    g1r_d = nc.dram_tensor("g1r", [1, D], DT, kind="ExternalInput")
    nb1r_d = nc.dram_tensor("nb1r", [1, D], DT, kind="ExternalInput")
    g2r_d = nc.dram_tensor("g2r", [1, D], DT, kind="ExternalInput")
    nb2r_d = nc.dram_tensor("nb2r", [1, D], DT, kind="ExternalInput")
    gamh_d = nc.dram_tensor("gamh", [P, TC, H], DT, kind="ExternalInput")
    out_d = nc.dram_tensor("outT", [P, DC, SCH], F32, kind="ExternalOutput")

    import contextlib
    lp = (nc.allow_low_precision(reason="float32r operands are rounded by design")
          if USE_F32R else contextlib.nullcontext())
    with lp, tile.TileContext(nc) as tc:
        with tc.tile_pool(name="small", bufs=1) as small, \
             tc.tile_pool(name="ps", bufs=2, space="PSUM") as pp, \
             tc.tile_pool(name="ps2", bufs=2, space="PSUM") as pp2, \
             tc.tile_pool(name="psacc", bufs=2, space="PSUM") as ppacc:

            # ---- constants (only V-phase-critical ones issued up front) ----
            onesw = small.tile([P, 512], DT)
            bq_sb = small.tile([P, DC], F32)
            bk_sb = small.tile([P, DC], F32)
            bo_sb = small.tile([P, DC], F32)
            bf1_sb = small.tile([P, FC], F32)
            bf2_sb = small.tile([P, DC], F32)
            gam_sb = small.tile([P, TC], F32)
            invd = small.tile([P, 1], DT)
            bv_row = small.tile([1, D], DT)
            ones = onesw[:, 0:P]
            epsc = small.tile([P, 1], F32)
            nc.vector.memset(epsc[:], EPS)
            nc.const_aps.aps[(F32, EPS)] = epsc[:]

            # long-lived tiles, allocated in reverse order of death (LIFO pools)
            hT, hT_free = tc.tile([P, DC, SCH], DT, name="hT")

            # ================= Phase V =================
            # v stored [token, feature] with a ones column per head (for Z).
            vA, vA_free = tc.tile([P, TC, H * WA], DT, name="vA")
            vA_h = vA[:].rearrange("p t (h c) -> p t h c", c=WA)
            # gamma column per head (Z weights; = mask gamma, 1.0 for unmasked)
            gamh_sb = small.tile([P, TC, H], DT)
            nc.sync.dma_start(gamh_sb[:], gamh_d[:])
            nc.vector.tensor_copy(vA_h[:, :, :, W], gamh_sb[:])
            with tc.tile_pool(name="pv", bufs=1) as pv, \
                 tc.tile_pool(name="pvw", bufs=4) as pvw:
                wv_sb = pv.tile([P, DC, D], DT)
                # first-needed data first: halves of wv[0] + first token window
                nc.sync.dma_start(wv_sb[:, 0, 0:512], wv_d[:, 0, 0:512])
                xws = {0: pvw.tile([P, DC, P], DT, tag="xw", name="xw0")}
                nc.scalar.dma_start(xws[0][:, 0:2], xT_d[:, 0:2, ts(0, P)])
                nc.scalar.dma_start(xws[0][:, 2:], xT_d[:, 2:, ts(0, P)])
                nc.sync.dma_start(wv_sb[:, 0, 512:], wv_d[:, 0, 512:])
                nc.sync.dma_start(gam_sb[:], gam_d[:])
                nc.sync.dma_start(bv_row[:], bv_d[:])
                nc.sync.dma_start(onesw[:], ones_d[:])
                nc.sync.dma_start(invd[:], invd_d[:])
                for dc in range(1, DC):
                    nc.sync.dma_start(wv_sb[:, dc], wv_d[:, dc])
                for sb, dr in [(bq_sb, bq_d), (bk_sb, bk_d), (bo_sb, bo_d),
                               (bf1_sb, bf1_d), (bf2_sb, bf2_d)]:
                    nc.sync.dma_start(sb[:], dr[:])
                for tcl in range(TC):
                    if tcl in xws:
                        xw = xws[tcl]
                    else:
                        xw = pvw.tile([P, DC, P], DT, tag="xw", name="xw")
                        eng = nc.scalar if tcl % 2 == 0 else nc.sync
                        eng.dma_start(xw[:], xT_d[:, :, ts(tcl, P)])
                    for dvh in range(2):
                        psv = (ppacc.tile([P, 512], F32, tag="acc", name="psv")
                               if dvh == 0 else
                               pp.tile([P, 512], F32, tag="ps", name="psv2"))
                        for dc in range(DC):
                            nc.tensor.matmul(psv[:], xw[:, dc],
                                             wv_sb[:, dc, ts(dvh, 512)],
                                             start=(dc == 0), stop=False)
                        nc.tensor.matmul(psv[:], ones[0:1, 0:P],
                                         bv_row[:, ts(dvh, 512)],
                                         start=False, stop=True)
                        nc.vector.tensor_scalar(
                            vA_h[:, tcl, dvh * 8:(dvh + 1) * 8, 0:W],
                            psv[:].rearrange("p (h c) -> p h c", c=W),
                            gam_sb[:, tcl:tcl + 1], None, mybir.AluOpType.mult,
                        )

            # ================= Phase K =================
            # kT stored [feature, token].
            kT, kT_free = tc.tile([P, DC, S], DT, name="kT")
            with tc.tile_pool(name="pk", bufs=1) as pk, \
                 tc.tile_pool(name="pkw", bufs=2) as pkw:
                wk_sb = pk.tile([P, DC, D], DT)
                nc.sync.dma_start(wk_sb[:, 0, 0:P], wk_d[:, 0, 0:P])
                nc.scalar.dma_start(wk_sb[:, 0, P:], wk_d[:, 0, P:])
                for dc in range(1, DC):
                    nc.sync.dma_start(wk_sb[:, dc], wk_d[:, dc])
                for tw in range(S // 256):
                    if False:
                        xw = None
                    else:
                        xw = pkw.tile([P, DC, 256], DT, tag="xw", name="xwk")
                        eng = nc.scalar if tw % 2 == 0 else nc.sync
                        eng.dma_start(xw[:], xT_d[:, :, ts(tw, 256)])
                    for dk in range(DC):
                        psk = pp.tile([P, 256], F32, tag="ps")
                        for dc in range(DC):
                            nc.tensor.matmul(psk[:], wk_sb[:, dc, ts(dk, P)],
                                             xw[:, dc],
                                             start=(dc == 0), stop=(dc == DC - 1))
                        nc.vector.tensor_scalar(kT[:, dk, ts(tw, 256)], psk[:],
                                                bk_sb[:, dk:dk + 1], None,
                                                mybir.AluOpType.add)

            # ================= Phase Q =================
            qT, qT_free = tc.tile([P, DC, SCH], DT, name="qT")
            with tc.tile_pool(name="pq", bufs=1) as pq, \
                 tc.tile_pool(name="pqw", bufs=3) as pqw:
                xs = pq.tile([P, DC, SCH], DT)
                for dc in range(DC):
                    nc.scalar.dma_start(xs[:, dc], xs_d[:, dc])
                for dq in range(DC):
                    wt = pqw.tile([P, DC, P], DT, tag="wt")
                    nc.sync.dma_start(wt[:], wq_d[:, :, ts(dq, P)])
                    psq = pp.tile([P, SCH], F32, tag="ps")
                    for dc in range(DC):
                        nc.tensor.matmul(psq[:], wt[:, dc], xs[:, dc],
                                         start=(dc == 0), stop=(dc == DC - 1))
                    nc.vector.tensor_scalar(qT[:, dq], psq[:],
                                            bq_sb[:, dq:dq + 1], None,
                                            mybir.AluOpType.add)

            # ================= Attention =================
            with tc.tile_pool(name="pat", bufs=1) as pat:
                for h in range(H):
                    hc, hp = h // 2, W * (h % 2)
                    pso = ppacc.tile([WA, SCH], F32, tag="acc")
                    for tcp in range(TC // 2):
                        pss = pp2.tile([P, 2 * SCH], F32, tag="ps2")
                        for j in range(2):
                            nc.tensor.matmul(pss[:, ts(j, SCH)],
                                             kT[hp:hp + W, hc, ts(2 * tcp + j, P)],
                                             qT[hp:hp + W, hc],
                                             start=True, stop=True)
                        probs = pat.tile([P, 2 * SCH], DT, tag="probs", bufs=4)
                        nc.scalar.activation(probs[:], pss[:], at.Exp,
                                             scale=float(SCALE))
                        for j in range(2):
                            tcl = 2 * tcp + j
                            nc.tensor.matmul(pso[:],
                                             vA[:, tcl, h * WA:(h + 1) * WA],
                                             probs[:, ts(j, SCH)],
                                             start=(tcl == 0), stop=(tcl == TC - 1))
                    rz = pat.tile([P, SCH], DT, tag="rz", bufs=2)
                    nc.vector.reciprocal(rz[W:W + 1], pso[W:W + 1])
                    psb = pp.tile([W, SCH], F32, tag="ps")
                    nc.tensor.matmul(psb[:], ones[W:W + 1, 0:W], rz[W:W + 1],
                                     start=True, stop=True)
                    rzb = pat.tile([W, SCH], DT, tag="rzb", bufs=2)
                    nc.vector.tensor_copy(rzb[:], psb[:])
                    if hp == 0:
                        nc.vector.tensor_tensor(hT[0:W, hc], pso[0:W], rzb[:],
                                                mybir.AluOpType.mult)
                    else:
                        tn = pat.tile([W, SCH], DT, tag="ntmp", bufs=2)
                        nc.vector.tensor_tensor(tn[:], pso[0:W], rzb[:],
                                                mybir.AluOpType.mult)
                        nc.sync.dma_start(hT[hp:hp + W, hc], tn[:])
            qT_free()
            kT_free()
            vA_free()

            # ================= Out-proj + residual =================
            # FFN-side tiles are allocated first so their SBUF slots do not
            # overlap the out-proj/LN1 scratch - lets w1/w2 DMAs prefetch
            # while LN1 is still running.
            prow_cm = tc.tile_pool(name="prow", bufs=1)
            prow = prow_cm.__enter__()
            g1r = prow.tile([1, D], DT)
            nb1r = prow.tile([1, D], DT)
            nc.scalar.dma_start(g1r[:], g1r_d[:])
            nc.scalar.dma_start(nb1r[:], nb1r_d[:])
            h1T, h1T_free = tc.tile([P, DC, SCH], DT, name="h1T")
            r2T, r2T_free = tc.tile([P, DC, SCH], DT, name="r2T")
            g1T, g1T_free = tc.tile([P, FC, SCH], DT, name="g1T")
            pf1_cm = tc.tile_pool(name="pf1", bufs=2)
            pf1 = pf1_cm.__enter__()
            r1T, r1T_free = tc.tile([P, DC, SCH], DT, name="r1T")
            sq1, sq1_free = tc.tile([P, DC, SCH], DT, name="sq1")
            with tc.tile_pool(name="po", bufs=1) as po, \
                 tc.tile_pool(name="pow", bufs=2) as pow_:
                xs2 = po.tile([P, DC, SCH], DT)
                wts = [pow_.tile([P, DC, P], DT, tag="wt", name=f"wo{dp}")
                       for dp in range(2)]
                nc.sync.dma_start(wts[0][:], wo_d[:, :, ts(0, P)])
                nc.scalar.dma_start(wts[1][:], wo_d[:, :, ts(1, P)])
                for dc in range(DC):
                    nc.scalar.dma_start(xs2[:, dc], xs_d[:, dc])
                # warm the Sqrt table while ACT is otherwise idle
                sqwarm = po.tile([1, 1], F32)
                nc.scalar.activation(sqwarm[:], epsc[0:1, :], at.Sqrt)
                w1t0 = pf1.tile([P, DC, 2 * P], DT, tag="wt", name="w1t0")
                nc.sync.dma_start(w1t0[:], w1_d[:, :, ts(0, 2 * P)])
                for dp in range(DC):
                    if dp < 2:
                        wt = wts[dp]
                    else:
                        wt = pow_.tile([P, DC, P], DT, tag="wt", name="wo")
                        eng = nc.sync if dp % 2 == 0 else nc.scalar
                        eng.dma_start(wt[:], wo_d[:, :, ts(dp, P)])
                    psr = pp.tile([P, SCH], F32, tag="ps")
                    for dc in range(DC):
                        nc.tensor.matmul(psr[:], wt[:, dc], hT[:, dc],
                                         start=(dc == 0), stop=(dc == DC - 1))
                    nc.vector.tensor_scalar(r1T[:, dp], psr[:],
                                            bo_sb[:, dp:dp + 1], None,
                                            mybir.AluOpType.add)
                    nc.vector.tensor_tensor(r1T[:, dp], r1T[:, dp], xs2[:, dp],
                                            mybir.AluOpType.add)
                    nc.scalar.activation(sq1[:, dp], r1T[:, dp], at.Square)

            # ================= LN1 =================
            _layer_norm(nc, tc, pp, pp2, ppacc, onesw, invd, r1T, sq1, h1T, g1r, nb1r, "ln1")
            sq1_free()
            r1T_free()
            # reuse the row tiles for LN2's affine rows
            nc.scalar.dma_start(g1r[:], g2r_d[:])
            nc.scalar.dma_start(nb1r[:], nb2r_d[:])
            sq2, sq2_free = tc.tile([P, DC, SCH], DT, name="sq2")

            # ================= FFN =================
            with tc.tile_pool(name="pf2", bufs=3) as pf2:
                for fcp in range(FC // 2):
                    if fcp == 0:
                        w1t = w1t0
                    else:
                        w1t = pf1.tile([P, DC, 2 * P], DT, tag="wt")
                        nc.sync.dma_start(w1t[:], w1_d[:, :, ts(fcp, 2 * P)])
                    for j in range(2):
                        fc = 2 * fcp + j
                        psg = pp.tile([P, SCH], F32, tag="ps")
                        for dc in range(DC):
                            nc.tensor.matmul(psg[:], w1t[:, dc, ts(j, P)],
                                             h1T[:, dc],
                                             start=(dc == 0), stop=(dc == DC - 1))
                        nc.scalar.activation(g1T[:, fc], psg[:], at.Gelu,
                                             bias=bf1_sb[:, fc:fc + 1])
                sqwarm2 = pf2.tile([1, 1], F32)
                nc.scalar.activation(sqwarm2[:], epsc[0:1, :], at.Sqrt)
                for dp in range(DC):
                    w2t = pf2.tile([P, FC, P], DT, tag="wt")
                    nc.sync.dma_start(w2t[:, 0:FC // 2], w2_d[:, 0:FC // 2, ts(dp, P)])
                    nc.sync.dma_start(w2t[:, FC // 2:], w2_d[:, FC // 2:, ts(dp, P)])
                    psf = ppacc.tile([P, SCH], F32, tag="acc")
                    for fc in range(FC):
                        nc.tensor.matmul(psf[:], w2t[:, fc], g1T[:, fc],
                                         start=(fc == 0), stop=(fc == FC - 1))
                    nc.vector.tensor_scalar(r2T[:, dp], psf[:],
                                            bf2_sb[:, dp:dp + 1], None,
                                            mybir.AluOpType.add)
                    nc.vector.tensor_tensor(r2T[:, dp], r2T[:, dp], h1T[:, dp],
                                            mybir.AluOpType.add)
                    nc.scalar.activation(sq2[:, dp], r2T[:, dp], at.Square)
            # ================= LN2 + out =================
            oT, oT_free = tc.tile([P, DC, SCH], F32, name="oT")
            _layer_norm(nc, tc, pp, pp2, ppacc, onesw, invd, r2T, sq2, oT, g1r, nb1r, "ln2")
            for dc in range(DC):
                nc.scalar.dma_start(out_d[:, dc], oT[:, dc])
            oT_free()
            sq2_free()
            pf1_cm.__exit__(None, None, None)
            g1T_free()
            r2T_free()
            h1T_free()
            prow_cm.__exit__(None, None, None)
            hT_free()

    nc.compile()
    return nc


def kernel(**inputs):
    x = np.asarray(inputs["x"], dtype=np.float32)
    mask = np.asarray(inputs["mask"])
    f = {k: np.asarray(inputs[k], dtype=np.float32) for k in
         ["wq", "bq", "wk", "bk", "wv", "bv", "wo", "bo", "g1", "b1",
          "w1", "bf1", "w2", "bf2", "g2", "b2"]}

    if "nc" not in _cache:
        _cache["nc"] = _build()
    nc = _cache["nc"]

    def wlay(w, pc):  # [K, M] -> [P, K//P, M]
        return np.ascontiguousarray(w.reshape(pc, P, w.shape[1]).transpose(1, 0, 2))

    def blay(b):      # [M] -> [P, M//P]
        return np.ascontiguousarray(b.reshape(-1, P).T)

    shared = {
        "wq": wlay(f["wq"], DC), "wk": wlay(f["wk"], DC), "wv": wlay(f["wv"], DC),
        "wo": wlay(f["wo"], DC), "w1": wlay(f["w1"], DC), "w2": wlay(f["w2"], FC),
        "ones_c": np.ones((P, 512), np.float32),
        "invd": np.full((P, 1), 1.0 / D, np.float32),
        "g1r": f["g1"].reshape(1, D), "g2r": f["g2"].reshape(1, D),
        "nb1r": (-f["b1"]).reshape(1, D),
        "nb2r": (-f["b2"]).reshape(1, D),
        "bq": blay(f["bq"]), "bk": blay(f["bk"]), "bvr": f["bv"].reshape(1, D),
        "bo": blay(f["bo"]), "bf1": blay(f["bf1"]), "bf2": blay(f["bf2"]),
        "g1": blay(f["g1"]), "b1": blay(f["b1"]),
        "g2": blay(f["g2"]), "b2": blay(f["b2"]),
    }

    in_maps = []
    for c in range(8):
        b, sq = c // 4, c % 4
        xTb = np.ascontiguousarray(x[b].T.reshape(DC, P, S).transpose(1, 0, 2))
        mbias = (-10000.0 * (1.0 - mask[b].astype(np.float32))).reshape(TC, P).T
        m = dict(shared)
        m["xT"] = xTb
        m["xs"] = np.ascontiguousarray(xTb[:, :, sq * SCH:(sq + 1) * SCH])
        gam = np.exp(mbias).astype(np.float32)          # 1.0 unmasked, 0.0 masked
        m["gam"] = np.ascontiguousarray(gam)
        m["gamh"] = np.ascontiguousarray(
            np.broadcast_to(gam[:, :, None], (P, TC, H)))
        in_maps.append(m)

    res = run_bass_kernel_spmd(nc, in_maps, core_ids=list(range(8)))
    _cache["last_res"] = res

    out = np.empty((B, S, D), np.float32)
    for c in range(8):
        b, sq = c // 4, c % 4
        oT = res.results[c]["outT"]  # [P, DC, SCH]
        out[b, sq * SCH:(sq + 1) * SCH, :] = oT.transpose(2, 1, 0).reshape(SCH, D)
    return out



# revision 37
# speedup vs baseline: 1.6905x; 1.6905x over previous
"""Trainium2 Bass kernel for a dense transformer encoder layer (fp8 edition).

Problem: B=2, S=2048, D=1024, H=16 heads (W=64), F=4096, fp32.

Sharding: 8 cores = 2 batches x 4 sequence chunks of 512 tokens. Each core
computes K/V for its batch's full sequence (replicated within its 4-core
batch group) and Q/attention/FFN for its own 512-token chunk. No collectives.

Speed comes from fp8(e4m3) DoubleRow matmuls: 256-deep contraction at 0.5
cycles/row = 4x the fp32r rate. All projections, attention-value, and both
FFN matmuls run in fp8-DR; QK^T scores stay plain fp8 (contraction is only
W=64). Residual stream / LayerNorm stay fp32. Accuracy is held to ~1.3e-2
rel-err by residual-compensated ("hi+lo") fp8 splits of h1, w1 and w2: the
FFN runs fc1 as 3 DR passes (hi*Whi + lo*Whi + hi*Wlo) and fc2 as 2
(g*W2hi + g*W2lo), which cancels the dominant quantization terms.

Softmax: probabilities are needed only in fp8, so most exp() calls never
touch the ACT engine: a single tensor_scalar (mult+add -> int8) computes
byte = trunc(score*8*log2e + 72.5), which IS the e4m3 bit pattern of
4*exp(score/8) (Schraudolph in fp8). This runs on DVE and GPSIMD in
parallel with real exp() on ACT (weighted round-robin), removing the ACT
bottleneck. The per-head normalizer Z rides a gamma column appended to V
(gamma = mask * 2^-5, so masked keys drop out of numerator and Z alike);
normalization multiplies by a DMA-partition-broadcast reciprocal row.

LayerNorm statistics ride 1/D-scaled ones-column matmuls and the affine
apply is rank-1 tiles built on the PE (as in the fp32r baseline). LN1's
gamma row is pre-scaled by 16 so h1 is stored 16x (better fp8 coverage);
fc2's descale and LN2's stats absorb the factor exactly.
"""
import numpy as np
import ml_dtypes
import concourse.bass as bass
from concourse import bacc
import concourse.mybir as mybir
import concourse.tile as tile
from concourse.bass import ts
from concourse.bass_utils import run_bass_kernel_spmd

P = 128
B, S, D, H, W, F = 2, 2048, 1024, 16, 64, 4096
DC = D // P            # 8 d-chunks
FC = F // P            # 32 f-chunks
FCP = FC // 2          # 16 f-chunk pairs
TC = S // P            # 16 key-token chunks
SCH = 512              # tokens per core
EPS = 1e-12
WA = W + 1             # per-head V columns incl. Z column

F32 = mybir.dt.float32
DT = mybir.dt.float32r   # for LN stats / bias-row matmuls
F8 = mybir.dt.float8e4   # e4m3, max normal 240
I8 = mybir.dt.int8
DR = mybir.MatmulPerfMode.DoubleRow
E4NP = ml_dtypes.float8_e4m3

# power-of-2 quantization scales (see fp8_sim.py validation: ~1.3e-2 rel err)
SX, SW, SQ, SK, SV, SP, SH, S1, SW1, SW2 = (
    16.0, 1024.0, 16.0, 16.0, 16.0, 4.0, 512.0, 16.0, 1024.0, 1024.0)
QK_DESC = SQ / (SX * SW)          # psum -> q units
O_DESC = 1.0 / (SH * SW)          # outproj psum -> h@wo units
G_DESC = 1.0 / (S1 * SW1)         # fc1 psum -> pre-gelu units
F_DESC = S1 / SW2                 # fc2 psum -> 16*ff units
ESC = 1.0 / (8.0 * SQ * SK)       # score psum -> scaled score (incl 1/sqrt(W))
LNSP = float(np.log(SP))
# Schraudolph-in-fp8: byte = trunc(pss*A8 + B8) is the e4m3 encoding of
# ~SP*exp(pss*ESC). 56 = bias(7)<<3, +8*log2(SP), +0.5 rounding recenter.
A8 = float(8.0 / np.log(2.0) * ESC)
B8 = float(56.0 + 8.0 * np.log2(SP) + 0.5)

_cache = {}


def _layer_norm(nc, tc, ppL, ppA, ppB, onesw, invd, src, sq, dst, grow, nbrow,
                tag, stats=None):
    """src/sq/dst: [P, DC, SCH] sbuf (feature on partitions). LN over features.
    sq = src*src from the producing phase. Apply: dst = src*A - B with rank-1
    A = g (x) rstd, B = g (x) u*rstd - b (x) 1 built on the PE. If the caller
    interleaved the stats matmuls into its producer loop, it passes the
    accumulated (ps_u, ps_v) via `stats`."""
    at = mybir.ActivationFunctionType
    with tc.tile_pool(name=tag, bufs=1) as pool:
        if stats is not None:
            ps_u, ps_v = stats
        else:
            ps_u = ppL.tile([1, SCH], F32, tag="psl")
            ps_v = ppL.tile([1, SCH], F32, tag="psl")
            for dc in range(DC):
                nc.tensor.matmul(ps_u[:], invd[:], src[:, dc],
                                 start=(dc == 0), stop=(dc == DC - 1))
            for dc in range(DC):
                nc.tensor.matmul(ps_v[:], invd[:], sq[:, dc],
                                 start=(dc == 0), stop=(dc == DC - 1))
        u = pool.tile([1, SCH], DT)
        rstd = pool.tile([1, SCH], DT)   # holds var -> sd -> rstd in place
        ur = pool.tile([1, SCH], DT)
        nc.vector.tensor_copy(u[:], ps_u[:])
        nc.vector.tensor_tensor(rstd[:], u[:], u[:], mybir.AluOpType.mult)
        nc.vector.tensor_tensor(rstd[:], ps_v[:], rstd[:], mybir.AluOpType.subtract)
        nc.scalar.activation(rstd[:], rstd[:], at.Sqrt, bias=EPS)
        nc.vector.reciprocal(rstd[:], rstd[:])
        nc.vector.tensor_tensor(ur[:], u[:], rstd[:], mybir.AluOpType.mult)
        for dc in range(DC):
            ps_a = ppA.tile([P, SCH], F32, tag="psa")
            ps_b = ppB.tile([P, SCH], F32, tag="psb")
            nc.tensor.matmul(ps_a[:], grow[:, ts(dc, P)], rstd[:],
                             start=True, stop=True)
            nc.tensor.matmul(ps_b[:], grow[:, ts(dc, P)], ur[:],
                             start=True, stop=False)
            nc.tensor.matmul(ps_b[:], nbrow[:, ts(dc, P)], onesw[0:1, 0:SCH],
                             start=False, stop=True)
            t = pool.tile([P, SCH], F32, tag="lnt", bufs=2)
            nc.vector.tensor_tensor(t[:], src[:, dc], ps_a[:],
                                    mybir.AluOpType.mult)
            nc.vector.tensor_tensor(dst[:, dc], t[:], ps_b[:],
                                    mybir.AluOpType.subtract)


def _build():
    at = mybir.ActivationFunctionType
    op = mybir.AluOpType
    nc = bacc.Bacc("TRN2", target_bir_lowering=False)

    xT8_d = nc.dram_tensor("xT8", [P, DC, S], F8, kind="ExternalInput")
    xs8_d = nc.dram_tensor("xs8", [P, DC, SCH], F8, kind="ExternalInput")
    xs2_d = nc.dram_tensor("xs2", [P, DC, SCH], F32, kind="ExternalInput")
    wq_d = nc.dram_tensor("wq8", [P, DC, D], F8, kind="ExternalInput")
    wk_d = nc.dram_tensor("wk8", [P, DC, D], F8, kind="ExternalInput")
    wv_d = nc.dram_tensor("wv8", [P, DC, D], F8, kind="ExternalInput")
    wo_d = nc.dram_tensor("wo8", [P, DC, D], F8, kind="ExternalInput")
    w1h_d = nc.dram_tensor("w1hi", [P, FCP, DC, 2 * P], F8, kind="ExternalInput")
    w1l_d = nc.dram_tensor("w1lo", [P, FCP, DC, 2 * P], F8, kind="ExternalInput")
    w2h_d = nc.dram_tensor("w2hi", [P, DC, FC, P], F8, kind="ExternalInput")
    w2l_d = nc.dram_tensor("w2lo", [P, DC, FC, P], F8, kind="ExternalInput")
    bvr_d = nc.dram_tensor("bvr", [1, D], DT, kind="ExternalInput")
    bq_d = nc.dram_tensor("bq", [P, DC], F32, kind="ExternalInput")
    bk_d = nc.dram_tensor("bk", [P, DC], F32, kind="ExternalInput")
    bo_d = nc.dram_tensor("bo", [P, DC], F32, kind="ExternalInput")
    bf1_d = nc.dram_tensor("bf1", [P, FC], F32, kind="ExternalInput")
    bf2_d = nc.dram_tensor("bf2s", [P, DC], F32, kind="ExternalInput")
    gam_d = nc.dram_tensor("gam", [P, TC], F32, kind="ExternalInput")
    gamh_d = nc.dram_tensor("gamh", [P, TC, H], F8, kind="ExternalInput")
    invd_d = nc.dram_tensor("invd", [P, 1], DT, kind="ExternalInput")
    ones_d = nc.dram_tensor("ones_c", [P, 512], DT, kind="ExternalInput")
    g1r_d = nc.dram_tensor("g1rs", [1, D], DT, kind="ExternalInput")
    nb1r_d = nc.dram_tensor("nb1rs", [1, D], DT, kind="ExternalInput")
    g2r_d = nc.dram_tensor("g2r", [1, D], DT, kind="ExternalInput")
    nb2r_d = nc.dram_tensor("nb2r", [1, D], DT, kind="ExternalInput")
    out_d = nc.dram_tensor("outT", [P, DC, SCH], F32, kind="ExternalOutput")

    with nc.allow_low_precision(reason="fp8 operands are quantized by design"), \
         tile.TileContext(nc) as tc:
        with tc.tile_pool(name="small", bufs=1) as small:
            # ---- small constants ----
            onesw = small.tile([P, 512], DT)
            bq_sb = small.tile([P, DC], F32)
            bk_sb = small.tile([P, DC], F32)
            bo_sb = small.tile([P, DC], F32)
            bf1_sb = small.tile([P, FC], F32)
            bf2_sb = small.tile([P, DC], F32)
            gam_sb = small.tile([P, TC], F32)
            invd = small.tile([P, 1], DT)
            bv_row = small.tile([1, D], DT)
            lnpc = small.tile([P, 1], F32)
            epsc = small.tile([P, 1], F32)
            g1r = small.tile([1, D], DT)   # reused for g2r after LN1
            nb1r = small.tile([1, D], DT)  # reused for nb2r after LN1
            nc.vector.memset(epsc[:], EPS)
            nc.vector.memset(lnpc[:], LNSP)
            nc.const_aps.aps[(F32, EPS)] = epsc[:]

            # ---- long-lived tiles (strict LIFO pool stack) ----
            hT, hT_free = tc.tile([P, DC, SCH], F8, name="hT")
            prow_cm = tc.tile_pool(name="prow", bufs=1)
            prow = prow_cm.__enter__()
            xs2 = prow.tile([P, DC, SCH], F32)
            wo_sb = prow.tile([P, DC, D], F8)
            vA, vA_free = tc.tile([P, TC, H * WA], F8, name="vA")
            kT, kT_free = tc.tile([P, DC, S], F8, name="kT")
            qT, qT_free = tc.tile([P, DC, SCH], F8, name="qT")
            vA_h = vA[:].rearrange("p t (h c) -> p t h c", c=WA)

            # gamma column per head (Z weights; = mask * SV/SH = 2^-5)
            gamh_sb = small.tile([P, TC, H], F8)

            # ================= Phase V + K (interleaved) =================
            with tc.tile_pool(name="pqkv", bufs=1) as pqkv, \
                 tc.tile_pool(name="psqk", bufs=4, space="PSUM") as psqk:
                wv_sb = pqkv.tile([P, DC, D], F8)
                wk_sb = pqkv.tile([P, DC, D], F8)
                wq_sb = pqkv.tile([P, DC, D], F8)
                xT8 = pqkv.tile([P, DC, S], F8)
                xs8 = pqkv.tile([P, DC, SCH], F8)
                # first-needed first
                nc.sync.dma_start(wv_sb[:, 0:2], wv_d[:, 0:2])
                nc.scalar.dma_start(xT8[:, :, ts(0, 512)], xT8_d[:, :, ts(0, 512)])
                nc.sync.dma_start(gamh_sb[:], gamh_d[:])
                nc.sync.dma_start(gam_sb[:], gam_d[:])
                nc.sync.dma_start(bv_row[:], bvr_d[:])
                nc.sync.dma_start(onesw[:], ones_d[:])
                nc.sync.dma_start(wv_sb[:, 2:], wv_d[:, 2:])
                nc.scalar.dma_start(wk_sb[:], wk_d[:])
                nc.sync.dma_start(bk_sb[:], bk_d[:])
                nc.sync.dma_start(invd[:], invd_d[:])
                nc.vector.tensor_copy(vA_h[:, :, :, W], gamh_sb[:])
                for c in range(1, 4):
                    nc.scalar.dma_start(xT8[:, :, ts(c, 512)],
                                        xT8_d[:, :, ts(c, 512)])
                nc.sync.dma_start(wq_sb[:], wq_d[:])
                nc.sync.dma_start(xs8[:], xs8_d[:])
                nc.sync.dma_start(bq_sb[:], bq_d[:])
                ones = onesw[:, 0:P]

                nv = 0
                nk = 0
                for c in range(4):
                    for tcl in range(4 * c, 4 * c + 4):
                        for dvh in range(2):
                            psv = psqk.tile([P, 512], F32, tag="ps", name="psv")
                            for i in range(4):
                                nc.tensor.matmul(
                                    psv[:], xT8[:, 2 * i:2 * i + 2, ts(tcl, P)],
                                    wv_sb[:, 2 * i:2 * i + 2, ts(dvh, 512)],
                                    start=(i == 0), stop=False, perf_mode=DR)
                            nc.tensor.matmul(psv[:], ones[0:1, 0:P],
                                             bv_row[:, ts(dvh, 512)],
                                             start=False, stop=True)
                            # GPSIMD cannot read PSUM; ACT Identity does the
                            # per-partition gamma scaling instead
                            if nv % 3 == 0:
                                nc.vector.tensor_scalar(
                                    vA_h[:, tcl, dvh * 8:(dvh + 1) * 8, 0:W],
                                    psv[:].rearrange("p (h c) -> p h c", c=W),
                                    gam_sb[:, tcl:tcl + 1], None, op.mult)
                            else:
                                nc.scalar.activation(
                                    vA_h[:, tcl, dvh * 8:(dvh + 1) * 8, 0:W],
                                    psv[:].rearrange("p (h c) -> p h c", c=W),
                                    at.Identity, scale=gam_sb[:, tcl:tcl + 1])
                            nv += 1
                    for tw in range(2 * c, 2 * c + 2):
                        for dk in range(DC):
                            psk = psqk.tile([P, 256], F32, tag="ps", name="psk")
                            for i in range(4):
                                nc.tensor.matmul(
                                    psk[:], wk_sb[:, 2 * i:2 * i + 2, ts(dk, P)],
                                    xT8[:, 2 * i:2 * i + 2, ts(tw, 256)],
                                    start=(i == 0), stop=(i == 3), perf_mode=DR)
                            if nk % 3 == 0:
                                nc.scalar.activation(kT[:, dk, ts(tw, 256)],
                                                     psk[:], at.Identity,
                                                     bias=bk_sb[:, dk:dk + 1],
                                                     scale=QK_DESC)
                            else:
                                nc.vector.tensor_scalar(kT[:, dk, ts(tw, 256)],
                                                        psk[:], QK_DESC,
                                                        bk_sb[:, dk:dk + 1],
                                                        op.mult, op.add)
                            nk += 1
                # ================= Phase Q =================
                for dq in range(DC):
                    psq = psqk.tile([P, 512], F32, tag="ps", name="psq")
                    for i in range(4):
                        nc.tensor.matmul(psq[:], wq_sb[:, 2 * i:2 * i + 2, ts(dq, P)],
                                         xs8[:, 2 * i:2 * i + 2, :],
                                         start=(i == 0), stop=(i == 3), perf_mode=DR)
                    if dq % 2:
                        nc.vector.tensor_scalar(qT[:, dq], psq[:], QK_DESC,
                                                bq_sb[:, dq:dq + 1],
                                                op.mult, op.add)
                    else:
                        nc.scalar.activation(qT[:, dq], psq[:], at.Identity,
                                             bias=bq_sb[:, dq:dq + 1],
                                             scale=QK_DESC)

            # ================= Attention =================
            # big prefetches for later phases ride under attention
            nc.sync.dma_start(bo_sb[:], bo_d[:])
            nc.sync.dma_start(bf1_sb[:], bf1_d[:])
            nc.sync.dma_start(bf2_sb[:], bf2_d[:])
            nc.sync.dma_start(g1r[:], g1r_d[:])
            nc.sync.dma_start(nb1r[:], nb1r_d[:])
            nc.scalar.dma_start(wo_sb[:], wo_d[:])
            for dc in range(DC):
                nc.scalar.dma_start(xs2[:, dc], xs2_d[:, dc])

            # probs engines: E = ACT exp, D = DVE byte-trick, B = SBUF-bounce
            # then GPSIMD byte-trick (GPSIMD cannot read PSUM). Weighted so
            # ACT/DVE/Pool all land under the PE's attention time.
            pattern = ['E', 'D', 'B', 'E', 'B', 'D', 'E', 'B',
                       'E', 'D', 'B', 'E', 'B', 'D', 'E', 'B']
            ecnt = 0
            ccnt = 0
            with tc.tile_pool(name="pat", bufs=1) as pat, \
                 tc.tile_pool(name="psS", bufs=3, space="PSUM") as psS, \
                 tc.tile_pool(name="psO", bufs=2, space="PSUM") as psO:
                # global score-pair stream with fixed lookahead over head
                # boundaries so PE never drains at a head switch
                tasks = [(h, tcp) for h in range(H) for tcp in range(TC // 2)]
                probs_tiles = {}
                emitted = [0]

                def emit_pairs(upto):
                    nonlocal ecnt, ccnt
                    while emitted[0] < min(upto, len(tasks)):
                        h, tcp = tasks[emitted[0]]
                        hc, hp = h // 2, W * (h % 2)
                        pss = psS.tile([P, 2 * SCH], F32, tag="pss", name="pss")
                        for j in range(2):
                            nc.tensor.matmul(
                                pss[:, ts(j, SCH)],
                                kT[hp:hp + W, hc, ts(2 * tcp + j, P)],
                                qT[hp:hp + W, hc], start=True, stop=True)
                        # produce probs NOW so the PSUM pair slot frees fast
                        # and SBUF tiles form the deep pipeline
                        probs = pat.tile([P, 2 * SCH], I8, tag="probs", bufs=6)
                        pf8 = probs[:].bitcast(F8)
                        kind = pattern[ecnt % len(pattern)]
                        ecnt += 1
                        if kind == 'E':
                            nc.scalar.activation(pf8[:], pss[:], at.Exp,
                                                 scale=ESC, bias=lnpc[:, 0:1])
                        elif kind == 'D':
                            nc.vector.tensor_scalar(probs[:], pss[:],
                                                    A8, B8, op.mult, op.add)
                        else:
                            stg = pat.tile([P, 2 * SCH], F32, tag="stg", bufs=3)
                            if ccnt % 2:
                                nc.scalar.activation(stg[:], pss[:], at.Copy)
                            else:
                                nc.vector.tensor_copy(stg[:], pss[:])
                            ccnt += 1
                            nc.gpsimd.tensor_scalar(probs[:], stg[:],
                                                    A8, B8, op.mult, op.add)
                        probs_tiles[(h, tcp)] = probs
                        emitted[0] += 1

                def finalize(fin):
                    nonlocal ccnt
                    h, pso = fin
                    hc, hp = h // 2, W * (h % 2)
                    rz = pat.tile([P, SCH], DT, tag="rz", bufs=2)
                    nc.vector.reciprocal(rz[W:W + 1], pso[W:W + 1])
                    psz = psS.tile([W, SCH], F32, tag="pss", name="psz")
                    nc.tensor.matmul(psz[:], onesw[W:W + 1, 0:W], rz[W:W + 1],
                                     start=True, stop=True)
                    rzb = pat.tile([W, SCH], DT, tag="rzb", bufs=2)
                    nc.scalar.activation(rzb[:], psz[:], at.Copy)
                    hA = pat.tile([W, SCH], F32, tag="hA", bufs=2)
                    if ccnt % 2:
                        nc.scalar.activation(hA[:], pso[0:W], at.Copy)
                    else:
                        nc.vector.tensor_copy(hA[:], pso[0:W])
                    ccnt += 1
                    if hp == 0:
                        nc.gpsimd.tensor_tensor(hT[0:W, hc], hA[:], rzb[:],
                                                op.mult)
                    else:
                        tn = pat.tile([W, SCH], F8, tag="ntmp", bufs=2)
                        nc.gpsimd.tensor_tensor(tn[:], hA[:], rzb[:], op.mult)
                        nc.sync.dma_start(hT[hp:hp + W, hc], tn[:])

                pending = None
                done = 0
                emit_pairs(3)
                for h in range(H):
                    pso = psO.tile([WA, SCH], F32, tag="pso")
                    for tcp in range(TC // 2):
                        probs = probs_tiles.pop((h, tcp))
                        pf8 = probs[:].bitcast(F8)
                        done += 1
                        nc.tensor.matmul(
                            pso[:], vA[:, 2 * tcp:2 * tcp + 2,
                                       h * WA:(h + 1) * WA],
                            pf8.rearrange("p (j t) -> p j t", j=2),
                            start=(tcp == 0), stop=(tcp == TC // 2 - 1),
                            perf_mode=DR)
                        if tcp == 1 and pending is not None:
                            finalize(pending)
                            pending = None
                        emit_pairs(done + 3)
                    pending = (h, pso)
                finalize(pending)
            qT_free()
            kT_free()
            vA_free()

            # ================= Out-proj + residual =================
            r2T, r2T_free = tc.tile([P, DC, SCH], DT, name="r2T")
            sq2, sq2_free = tc.tile([P, DC, SCH], DT, name="sq2")
            h1T, h1T_free = tc.tile([P, DC, SCH], DT, name="h1T")
            pf1_cm = tc.tile_pool(name="pf1", bufs=2)
            pf1 = pf1_cm.__enter__()
            w1t0h = pf1.tile([P, DC, 2 * P], F8, tag="wh", name="w1t0h")
            w1t0l = pf1.tile([P, DC, 2 * P], F8, tag="wl", name="w1t0l")
            nc.sync.dma_start(w1t0h[:], w1h_d[:, 0])
            nc.sync.dma_start(w1t0l[:], w1l_d[:, 0])
            r1T, r1T_free = tc.tile([P, DC, SCH], DT, name="r1T")
            sq1, sq1_free = tc.tile([P, DC, SCH], DT, name="sq1")
            with tc.tile_pool(name="ppo", bufs=2, space="PSUM") as ppo, \
                 tc.tile_pool(name="ppL", bufs=2, space="PSUM") as ppL, \
                 tc.tile_pool(name="ppA", bufs=2, space="PSUM") as ppA, \
                 tc.tile_pool(name="ppB", bufs=2, space="PSUM") as ppB:
                # LN1 stats matmuls ride inside the producer loop, a few dp
                # behind the elementwise chain so PE never waits on it.
                ps_u1 = ppL.tile([1, SCH], F32, tag="psl")
                ps_v1 = ppL.tile([1, SCH], F32, tag="psl")
                for dp in range(DC + 3):
                    if dp < DC:
                        psr = ppo.tile([P, SCH], F32, tag="ps")
                        for i in range(4):
                            nc.tensor.matmul(psr[:],
                                             wo_sb[:, 2 * i:2 * i + 2, ts(dp, P)],
                                             hT[:, 2 * i:2 * i + 2, :],
                                             start=(i == 0), stop=(i == 3),
                                             perf_mode=DR)
                        nc.vector.tensor_scalar(r1T[:, dp], psr[:], O_DESC,
                                                bo_sb[:, dp:dp + 1],
                                                op.mult, op.add)
                        nc.gpsimd.tensor_tensor(r1T[:, dp], r1T[:, dp],
                                                xs2[:, dp], op.add)
                        nc.scalar.activation(sq1[:, dp], r1T[:, dp], at.Square)
                    if 2 <= dp < DC + 2:
                        d = dp - 2
                        nc.tensor.matmul(ps_u1[:], invd[:], r1T[:, d],
                                         start=(d == 0), stop=(d == DC - 1))
                    if 3 <= dp:
                        d = dp - 3
                        nc.tensor.matmul(ps_v1[:], invd[:], sq1[:, d],
                                         start=(d == 0), stop=(d == DC - 1))

                # ================= LN1 (dst = 16*ln1 via scaled g1 row) ====
                _layer_norm(nc, tc, ppL, ppA, ppB, onesw, invd, r1T, sq1, h1T,
                            g1r, nb1r, "ln1", stats=(ps_u1, ps_v1))
            sq1_free()
            r1T_free()
            pf2_cm = tc.tile_pool(name="pf2", bufs=3)
            pf2 = pf2_cm.__enter__()
            h1hi, h1hi_free = tc.tile([P, DC, SCH], F8, name="h1hi")
            h1lo, h1lo_free = tc.tile([P, DC, SCH], F8, name="h1lo")
            g1T, g1T_free = tc.tile([P, FC, SCH], F8, name="g1T")
            oT, oT_free = tc.tile([P, DC, SCH], F32, name="oT")
            # rows for LN2 reuse LN1's tiles
            nc.sync.dma_start(g1r[:], g2r_d[:])
            nc.sync.dma_start(nb1r[:], nb2r_d[:])
            # head-start on fc2 weights
            w2ts = {}
            for dp in range(2):
                w2th = pf2.tile([P, FC, P], F8, tag="wh", name=f"w2h{dp}")
                w2tl = pf2.tile([P, FC, P], F8, tag="wl", name=f"w2l{dp}")
                nc.sync.dma_start(w2th[:], w2h_d[:, dp])
                nc.sync.dma_start(w2tl[:], w2l_d[:, dp])
                w2ts[dp] = (w2th, w2tl)
            for dc in range(DC):
                nc.gpsimd.tensor_copy(h1hi[:, dc], h1T[:, dc])
                nc.gpsimd.tensor_tensor(h1lo[:, dc], h1T[:, dc], h1hi[:, dc],
                                        op.subtract)

            # ================= FFN =================
            ppL2_cm = tc.tile_pool(name="ppL2", bufs=2, space="PSUM")
            ppL2 = ppL2_cm.__enter__()
            with tc.tile_pool(name="ppf", bufs=4, space="PSUM") as ppf:
                for fcp in range(FCP):
                    if fcp == 0:
                        w1th, w1tl = w1t0h, w1t0l
                    else:
                        w1th = pf1.tile([P, DC, 2 * P], F8, tag="wh")
                        w1tl = pf1.tile([P, DC, 2 * P], F8, tag="wl")
                        nc.sync.dma_start(w1th[:], w1h_d[:, fcp])
                        nc.sync.dma_start(w1tl[:], w1l_d[:, fcp])
                    for j in range(2):
                        fc = 2 * fcp + j
                        psg = ppf.tile([P, SCH], F32, tag="ps")
                        for i in range(4):
                            nc.tensor.matmul(
                                psg[:], w1th[:, 2 * i:2 * i + 2, ts(j, P)],
                                h1hi[:, 2 * i:2 * i + 2, :],
                                start=(i == 0), stop=False, perf_mode=DR)
                        for i in range(4):
                            nc.tensor.matmul(
                                psg[:], w1th[:, 2 * i:2 * i + 2, ts(j, P)],
                                h1lo[:, 2 * i:2 * i + 2, :],
                                start=False, stop=False, perf_mode=DR)
                        for i in range(4):
                            nc.tensor.matmul(
                                psg[:], w1tl[:, 2 * i:2 * i + 2, ts(j, P)],
                                h1hi[:, 2 * i:2 * i + 2, :],
                                start=False, stop=(i == 3), perf_mode=DR)
                        nc.scalar.activation(g1T[:, fc], psg[:], at.Gelu,
                                             bias=bf1_sb[:, fc:fc + 1],
                                             scale=G_DESC)
                ps_u2 = ppL2.tile([1, SCH], F32, tag="psl")
                ps_v2 = ppL2.tile([1, SCH], F32, tag="psl")
                for dp in range(DC + 3):
                    if dp < DC:
                        if dp in w2ts:
                            w2th, w2tl = w2ts.pop(dp)
                        else:
                            w2th = pf2.tile([P, FC, P], F8, tag="wh")
                            w2tl = pf2.tile([P, FC, P], F8, tag="wl")
                            nc.sync.dma_start(w2th[:], w2h_d[:, dp])
                            nc.sync.dma_start(w2tl[:], w2l_d[:, dp])
                        psf = ppf.tile([P, SCH], F32, tag="ps")
                        for f in range(FC // 2):
                            nc.tensor.matmul(psf[:], w2th[:, 2 * f:2 * f + 2, :],
                                             g1T[:, 2 * f:2 * f + 2, :],
                                             start=(f == 0), stop=False,
                                             perf_mode=DR)
                        for f in range(FC // 2):
                            nc.tensor.matmul(psf[:], w2tl[:, 2 * f:2 * f + 2, :],
                                             g1T[:, 2 * f:2 * f + 2, :],
                                             start=False, stop=(f == FC // 2 - 1),
                                             perf_mode=DR)
                        nc.vector.tensor_scalar(r2T[:, dp], psf[:], F_DESC,
                                                bf2_sb[:, dp:dp + 1],
                                                op.mult, op.add)
                        nc.gpsimd.tensor_tensor(r2T[:, dp], r2T[:, dp],
                                                h1T[:, dp], op.add)
                        nc.scalar.activation(sq2[:, dp], r2T[:, dp], at.Square)
                    if 2 <= dp < DC + 2:
                        d = dp - 2
                        nc.tensor.matmul(ps_u2[:], invd[:], r2T[:, d],
                                         start=(d == 0), stop=(d == DC - 1))
                    if 3 <= dp:
                        d = dp - 3
                        nc.tensor.matmul(ps_v2[:], invd[:], sq2[:, d],
                                         start=(d == 0), stop=(d == DC - 1))
            # ========== LN2 (input is 16*(h1+ff); scale cancels) + out =====
            with tc.tile_pool(name="ppA2", bufs=2, space="PSUM") as ppA2, \
                 tc.tile_pool(name="ppB2", bufs=2, space="PSUM") as ppB2:
                _layer_norm(nc, tc, ppL2, ppA2, ppB2, onesw, invd, r2T, sq2, oT,
                            g1r, nb1r, "ln2", stats=(ps_u2, ps_v2))
            ppL2_cm.__exit__(None, None, None)
            for dc in range(DC):
                nc.sync.dma_start(out_d[:, dc], oT[:, dc])
            # pops in exact reverse push order
            oT_free()
            g1T_free()
            h1lo_free()
            h1hi_free()
            pf2_cm.__exit__(None, None, None)
            pf1_cm.__exit__(None, None, None)
            h1T_free()
            sq2_free()
            r2T_free()
            prow_cm.__exit__(None, None, None)
            hT_free()

    nc.compile()
    return nc


def kernel(**inputs):
    x = np.asarray(inputs["x"], dtype=np.float32)
    mask = np.asarray(inputs["mask"])
    f = {k: np.asarray(inputs[k], dtype=np.float32) for k in
         ["wq", "bq", "wk", "bk", "wv", "bv", "wo", "bo", "g1", "b1",
          "w1", "bf1", "w2", "bf2", "g2", "b2"]}

    if "nc" not in _cache:
        _cache["nc"] = _build()
    nc = _cache["nc"]

    def wlay(w, pc):  # [K, M] -> [P, K//P, M]
        return np.ascontiguousarray(w.reshape(pc, P, w.shape[1]).transpose(1, 0, 2))

    def blay(b):      # [M] -> [P, M//P]
        return np.ascontiguousarray(b.reshape(-1, P).T)

    def q8(a):
        return np.ascontiguousarray(a).astype(E4NP)

    w1s = wlay(f["w1"] * SW1, DC)                      # [P, DC, F] scaled
    w1hi = w1s.astype(E4NP).astype(np.float32)
    w1lo = (w1s - w1hi).astype(E4NP)
    w1hi = w1hi.astype(E4NP)
    # -> [P, FCP, DC, 256]
    def w1tile(w):
        return np.ascontiguousarray(
            w.reshape(P, DC, FCP, 2 * P).transpose(0, 2, 1, 3))
    w2s = wlay(f["w2"] * SW2, FC)                      # [P, FC, D]
    w2hi = w2s.astype(E4NP).astype(np.float32)
    w2lo = (w2s - w2hi).astype(E4NP)
    w2hi = w2hi.astype(E4NP)
    def w2tile(w):
        return np.ascontiguousarray(
            w.reshape(P, FC, DC, P).transpose(0, 2, 1, 3))

    shared = {
        "wq8": q8(wlay(f["wq"] * SW, DC)), "wk8": q8(wlay(f["wk"] * SW, DC)),
        "wv8": q8(wlay(f["wv"] * SW, DC)), "wo8": q8(wlay(f["wo"] * SW, DC)),
        "w1hi": w1tile(w1hi), "w1lo": w1tile(w1lo),
        "w2hi": w2tile(w2hi), "w2lo": w2tile(w2lo),
        "ones_c": np.ones((P, 512), np.float32),
        "invd": np.full((P, 1), 1.0 / D, np.float32),
        "g1rs": (f["g1"] * S1).reshape(1, D),
        "nb1rs": (-f["b1"] * S1).reshape(1, D),
        "g2r": f["g2"].reshape(1, D), "nb2r": (-f["b2"]).reshape(1, D),
        "bq": blay(f["bq"] * SQ), "bk": blay(f["bk"] * SK),
        "bvr": (f["bv"] * SX * SW).reshape(1, D),
        "bo": blay(f["bo"]), "bf1": blay(f["bf1"]),
        "bf2s": blay(f["bf2"] * S1),
    }

    in_maps = []
    for c in range(8):
        b, sq = c // 4, c % 4
        xT = np.ascontiguousarray(x[b].T.reshape(DC, P, S).transpose(1, 0, 2))
        gamma = np.exp(-10000.0 * (1.0 - mask[b].astype(np.float32)))
        gamma = gamma.reshape(TC, P).T                  # [P, TC], 1.0/0.0
        m = dict(shared)
        m["xT8"] = q8(xT * SX)
        m["xs8"] = q8(xT[:, :, sq * SCH:(sq + 1) * SCH] * SX)
        m["xs2"] = np.ascontiguousarray(xT[:, :, sq * SCH:(sq + 1) * SCH])
        m["gam"] = np.ascontiguousarray(gamma / (SX * SW / SV))
        m["gamh"] = q8(np.broadcast_to(
            (gamma * (SV / SH))[:, :, None], (P, TC, H)))
        in_maps.append(m)

    res = run_bass_kernel_spmd(nc, in_maps, core_ids=list(range(8)))
    _cache["last_res"] = res

    out = np.empty((B, S, D), np.float32)
    for c in range(8):
        b, sq = c // 4, c % 4
        oT = res.results[c]["outT"]  # [P, DC, SCH]
        out[b, sq * SCH:(sq + 1) * SCH, :] = oT.transpose(2, 1, 0).reshape(SCH, D)
    return out
